# revision 1
# baseline (speedup 1.0000x reference)
"""GAT multi-head block on 8 Trainium2 NeuronCores.

Edge-parallel, dst-sharded. Host sorts edges by dst, shards dst ranges
across cores (98 blocks x 128 nodes each), groups each block's edges by
src int16-window (dma_gather index limit) and pads each (block, window)
run to T_W tiles of 128 edges. Fused host weights:
  Wa = [W_h@att_src_h | W_h@att_dst_h] [64,8], WWl_h = W_h@Wl_h, blp = bias@Wl+bl.
Device: phase A computes A[n] = x[n]@Wa, writing a_src into the gather
table rows (x_ext cols 66:70) and a_dst into A_tab. Phase B per tile:
dma_gather x_ext rows ([x|1|asrc] 256B bf16), gather block a_dst rows,
u = exp(leakyrelu(asrc+adst)), one-hot eq[e,v]=(dst_local==v) via
is_equal vs iota, rhs = concat_h(u_h*[x|1]), one PE matmul per tile
accumulates per-head weighted x-sums + softmax denominators in PSUM.
Per block: normalize, transpose, apply fused WWl_h + bias, write out.
"""

import os
import sys
import numpy as np

for _p in ("/opt/trn_rl_repo",):
    if _p not in sys.path:
        sys.path.insert(0, _p)

import concourse.bass as bass
import concourse.bacc as bacc
import concourse.mybir as mybir
import concourse.tile as tile

F32 = mybir.dt.float32
BF16 = mybir.dt.bfloat16
I16 = mybir.dt.int16
NP_BF16 = np.dtype(mybir.dt.np(BF16))

NEG_SLOPE = 0.2
P = 128
N_CORES = 8
WIN = 32768          # dma_gather int16 index window (rows)
XW = 128             # x_ext row width (256B bf16): [x(64) | 1 | 0 | asrc(4) | pad]
ASRC_COL = 66


def _ap(t, offset_elems, dims):
    return bass.AP(t, offset_elems, [list(d) for d in dims])


def build_program(N_PAD, BLOCKS, T_W, D, H):
    SKIP_GATHER = bool(os.environ.get("BASS_GAT_SKIP_GATHER"))
    SKIP_COMPUTE = bool(os.environ.get("BASS_GAT_SKIP_COMPUTE"))
    n_win = (N_PAD + WIN - 1) // WIN
    TPB = n_win * T_W                  # tiles per block
    TILES = BLOCKS * TPB
    RW = H * (D + 1)                   # 260
    n_ch = H * D // P

    nc = bacc.Bacc("TRN2", target_bir_lowering=False, debug=False,
                   num_devices=N_CORES)

    xT = nc.declare_dram_parameter("xT", [D, N_PAD], BF16, isOutput=False)
    xTd = nc.declare_dram_parameter("xTd", [D, BLOCKS * P], BF16, isOutput=False)
    n_win_tmp = (N_PAD + WIN - 1) // WIN
    x_exts = [
        nc.declare_dram_parameter(f"x_ext{w}",
                                  [min(WIN, N_PAD - w * WIN), XW], BF16,
                                  isOutput=False)
        for w in range(n_win_tmp)
    ]
    Wa = nc.declare_dram_parameter("Wa", [D, 2 * H], BF16, isOutput=False)
    WWl = nc.declare_dram_parameter("WWl", [P, n_ch * D], BF16, isOutput=False)
    blp = nc.declare_dram_parameter("blp", [1, D], BF16, isOutput=False)
    ident = nc.declare_dram_parameter("ident", [P, P], BF16, isOutput=False)
    iota = nc.declare_dram_parameter("iota", [P, P], BF16, isOutput=False)
    ones_r = nc.declare_dram_parameter("ones_r", [1, P], BF16, isOutput=False)
    src16 = nc.declare_dram_parameter("src16", [P, TILES * 8], I16, isOutput=False)
    dst16 = nc.declare_dram_parameter("dst16", [P, TILES * 8], I16, isOutput=False)
    dstloc = nc.declare_dram_parameter("dstloc", [P, TILES], BF16, isOutput=False)
    out = nc.declare_dram_parameter("out", [BLOCKS * P, D], F32, isOutput=True)

    A_loc = nc.dram_tensor("A_loc", [BLOCKS * P, XW], BF16)  # [adst(4) | junk]

    NT = N_PAD // P
    A_SLAB = 64
    n_slabs = (NT + A_SLAB - 1) // A_SLAB

    with tile.TileContext(nc) as tc:
        with tc.tile_pool(name="const", bufs=1) as cpool:
            wa_sb = cpool.tile([D, 2 * H], BF16, tag="wa")
            nc.sync.dma_start(out=wa_sb[:], in_=Wa[:])
            iota_sb = cpool.tile([P, P], BF16, tag="iota")
            nc.sync.dma_start(out=iota_sb[:], in_=iota[:])
            ident_sb = cpool.tile([P, P], BF16, tag="ident")
            nc.sync.dma_start(out=ident_sb[:], in_=ident[:])
            wwl_sb = cpool.tile([P, n_ch * D], BF16, tag="wwl")
            nc.sync.dma_start(out=wwl_sb[:], in_=WWl[:])
            blp_sb = cpool.tile([1, D], BF16, tag="blp")
            nc.sync.dma_start(out=blp_sb[:], in_=blp[:])
            ones_sb = cpool.tile([1, P], BF16, tag="ones")
            nc.sync.dma_start(out=ones_sb[:], in_=ones_r[:])
            src16_sb = cpool.tile([P, TILES * 8], I16, tag="src16")
            nc.sync.dma_start(out=src16_sb[:], in_=src16[:])
            dst16_sb = cpool.tile([P, TILES * 8], I16, tag="dst16")
            nc.sync.dma_start(out=dst16_sb[:], in_=dst16[:])
            dstloc_sb = cpool.tile([P, TILES], BF16, tag="dstloc")
            nc.sync.dma_start(out=dstloc_sb[:], in_=dstloc[:])

            # ---------- phase A:  [a_src | a_dst] = x @ Wa ----------
            with (
                tc.tile_pool(name="a_xt", bufs=2) as xt_pool,
                tc.tile_pool(name="a_ps", bufs=4, space="PSUM") as aps_pool,
                tc.tile_pool(name="a_st", bufs=2) as ast_pool,
            ):
                for s in range(n_slabs):
                    t0 = s * A_SLAB
                    nt = min(A_SLAB, NT - t0)
                    slab = xt_pool.tile([D, A_SLAB * P], BF16, tag="slab")
                    nc.sync.dma_start(out=slab[:, : nt * P],
                                      in_=xT[:, t0 * P:(t0 + nt) * P])
                    stage = ast_pool.tile([P, A_SLAB, 2 * H], BF16, tag="ast")
                    for t in range(nt):
                        aps = aps_pool.tile([P, 2 * H], F32, space="PSUM",
                                            tag="aps")
                        nc.tensor.matmul(aps[:], slab[:, t * P:(t + 1) * P],
                                         wa_sb[:], start=True, stop=True)
                        nc.any.tensor_copy(out=stage[:, t, :], in_=aps[:])
                    # a_src -> x_ext{w}[:, ASRC_COL:ASRC_COL+4]
                    w = (t0 * P) // WIN
                    r0 = t0 * P - w * WIN
                    nc.sync.dma_start(
                        out=_ap(x_exts[w], r0 * XW + ASRC_COL,
                                [[XW, P], [P * XW, nt], [1, H]]),
                        in_=stage[:, :nt, 0:H])
                # phase A2: a_dst for this core's own dst range -> A_loc
                NTd = BLOCKS * P // P
                n_slabs_d = (NTd + A_SLAB - 1) // A_SLAB
                for s in range(n_slabs_d):
                    t0 = s * A_SLAB
                    nt = min(A_SLAB, NTd - t0)
                    slab = xt_pool.tile([D, A_SLAB * P], BF16, tag="slab")
                    nc.sync.dma_start(out=slab[:, : nt * P],
                                      in_=xTd[:, t0 * P:(t0 + nt) * P])
                    staged = ast_pool.tile([P, A_SLAB, XW], BF16, tag="astd")
                    nc.vector.memset(staged[:], 0.0)
                    for t in range(nt):
                        aps = aps_pool.tile([P, 2 * H], F32, space="PSUM",
                                            tag="aps")
                        nc.tensor.matmul(aps[:], slab[:, t * P:(t + 1) * P],
                                         wa_sb[:], start=True, stop=True)
                        nc.any.tensor_copy(out=staged[:, t, 0:H],
                                           in_=aps[:, H:2 * H])
                    nc.sync.dma_start(
                        out=_ap(A_loc, t0 * P * XW,
                                [[XW, P], [P * XW, nt], [1, XW]]),
                        in_=staged[:, :nt, :])

            # ---------- phase B ----------
            with (
                tc.tile_pool(name="gx", bufs=2) as gx_pool,
                tc.tile_pool(name="ag", bufs=2) as ag_pool,
                tc.tile_pool(name="uexp", bufs=2) as u_pool,
                tc.tile_pool(name="eq", bufs=4) as eq_pool,
                tc.tile_pool(name="rhs", bufs=4) as rhs_pool,
                tc.tile_pool(name="m1", bufs=2, space="PSUM") as m1_pool,
                tc.tile_pool(name="post_ps", bufs=2, space="PSUM") as pps_pool,
                tc.tile_pool(name="post_sb", bufs=3) as psb_pool,
                tc.tile_pool(name="fout", bufs=2) as fout_pool,
            ):
                for b in range(BLOCKS):
                    g0 = b * TPB
                    gxb = gx_pool.tile([P, TPB, XW], BF16, tag="gx")
                    GCH = 2                      # tiles per gather call
                    if b == 0 or not SKIP_GATHER:
                        for w in range(n_win):
                            for q0 in range(0, T_W, GCH):
                                qn = min(GCH, T_W - q0)
                                tq = w * T_W + q0
                                gq = g0 + tq
                                nc.gpsimd.dma_gather(
                                    gxb[:, tq:tq + qn, :],
                                    x_exts[w][:, :],
                                    src16_sb[:, gq * 8:(gq + qn) * 8],
                                    qn * P, qn * P, XW, single_packet=False)
                    agD = ag_pool.tile([P, TPB, XW], BF16, tag="agD")
                    if b == 0 or not SKIP_GATHER:
                        for q0 in range(0, TPB, GCH):
                            qn = min(GCH, TPB - q0)
                            nc.gpsimd.dma_gather(
                                agD[:, q0:q0 + qn, :], A_loc[:, :],
                                dst16_sb[:, (g0 + q0) * 8:(g0 + q0 + qn) * 8],
                                qn * P, qn * P, XW, single_packet=False)
                    if SKIP_COMPUTE and b > 0:
                        continue
                    # u = exp(leaky_relu(asrc + adst)) for the whole block
                    lg = u_pool.tile([P, TPB, H], F32, tag="lg")
                    nc.vector.tensor_add(
                        out=lg[:],
                        in0=_ap(gxb.tensor, gxb.offset + ASRC_COL,
                                [list(gxb.ap[0]), [XW, TPB], [1, H]]),
                        in1=_ap(agD.tensor, agD.offset,
                                [list(agD.ap[0]), [XW, TPB], [1, H]]))
                    lr = u_pool.tile([P, TPB, H], F32, tag="lr")
                    nc.vector.scalar_tensor_tensor(
                        out=lr[:], in0=lg[:], scalar=NEG_SLOPE, in1=lg[:],
                        op0=mybir.AluOpType.mult, op1=mybir.AluOpType.max)
                    ue = u_pool.tile([P, TPB, H], BF16, tag="ue")
                    nc.scalar.activation(out=ue[:], in_=lr[:],
                                         func=mybir.ActivationFunctionType.Exp)

                    m1_ps = m1_pool.tile([P, RW], F32, space="PSUM", tag="m1")
                    for t in range(TPB):
                        g = g0 + t
                        eq = eq_pool.tile([P, P], BF16, tag="eq")
                        nc.vector.tensor_tensor(
                            out=eq[:],
                            in0=dstloc_sb[:, g:g + 1].to_broadcast([P, P]),
                            in1=iota_sb[:], op=mybir.AluOpType.is_equal)
                        rhs = rhs_pool.tile([P, RW], BF16, tag="rhs")
                        nc.vector.tensor_mul(
                            out=_ap(rhs.tensor, rhs.offset,
                                    [list(rhs.ap[0]), [D + 1, H], [1, D + 1]]),
                            in0=_ap(gxb.tensor, gxb.offset + t * XW,
                                    [list(gxb.ap[0]), [0, H], [1, D + 1]]),
                            in1=_ap(ue.tensor, ue.offset + t * H,
                                    [list(ue.ap[0]), [1, H], [0, D + 1]]))
                        nc.tensor.matmul(m1_ps[:], eq[:], rhs[:],
                                         start=(t == 0), stop=(t == TPB - 1))

                    # ---- block post ----
                    m1_t = m1_ps.tensor
                    rcp = psb_pool.tile([P, H], F32, tag="rcp")
                    nc.vector.tensor_scalar_add(
                        out=rcp[:],
                        in0=_ap(m1_t, m1_ps.offset + D,
                                [list(m1_ps.ap[0]), [D + 1, H]]),
                        scalar1=1e-16)
                    nc.vector.reciprocal(out=rcp[:], in_=rcp[:])
                    m1n = psb_pool.tile([P, H * D], BF16, tag="m1n")
                    nc.vector.tensor_mul(
                        out=_ap(m1n.tensor, m1n.offset,
                                [list(m1n.ap[0]), [D, H], [1, D]]),
                        in0=_ap(m1_t, m1_ps.offset,
                                [list(m1_ps.ap[0]), [D + 1, H], [1, D]]),
                        in1=_ap(rcp.tensor, rcp.offset,
                                [list(rcp.ap[0]), [1, H], [0, D]]))
                    f_ps = pps_pool.tile([P, D], F32, space="PSUM", tag="fps")
                    for ch in range(n_ch):
                        tp = pps_pool.tile([P, P], BF16, space="PSUM", tag="tp")
                        nc.tensor.transpose(
                            tp[:], m1n[:, ch * P:(ch + 1) * P], ident_sb[:])
                        tps = psb_pool.tile([P, P], BF16, tag="tps")
                        nc.any.tensor_copy(out=tps[:], in_=tp[:])
                        nc.tensor.matmul(f_ps[:], tps[:],
                                         wwl_sb[:, ch * D:(ch + 1) * D],
                                         start=(ch == 0), stop=False)
                    nc.tensor.matmul(f_ps[:], ones_sb[:], blp_sb[:],
                                     start=False, stop=True)
                    f_sb = fout_pool.tile([P, D], F32, tag="fsb")
                    nc.any.tensor_copy(out=f_sb[:], in_=f_ps[:])
                    nc.sync.dma_start(out=out[b * P:(b + 1) * P, :], in_=f_sb[:])

    nc.compile()
    return nc


def _wrap16(vals):
    """[n*128] int -> [128, n*8] int16 in dma_gather wrapped-replicated layout."""
    n = len(vals) // P
    a = np.asarray(vals, np.int16).reshape(n, 8, 16)     # i = t*128 + c*16 + p
    a = a.transpose(2, 0, 1).reshape(16, n * 8)          # [16, n*8]
    return np.tile(a, (8, 1))                            # replicate to 128


def _host_prep(x, edge_index, W, att_src, att_dst, bias, Wl, bl):
    N, D = x.shape
    H = att_src.shape[0]

    NBLK_TOTAL = (N + P - 1) // P
    BLOCKS = (NBLK_TOTAL + N_CORES - 1) // N_CORES
    N_PAD = max(BLOCKS * N_CORES, NBLK_TOTAL) * P
    if N_PAD <= N:
        N_PAD += P
    n_win = (N_PAD + WIN - 1) // WIN

    Wf = np.asarray(W, np.float64)
    Wlf = np.asarray(Wl, np.float64)
    Was = np.stack([Wf[:, h * D:(h + 1) * D] @ np.asarray(att_src[h], np.float64)
                    for h in range(H)], axis=1)
    Wad = np.stack([Wf[:, h * D:(h + 1) * D] @ np.asarray(att_dst[h], np.float64)
                    for h in range(H)], axis=1)
    Wa = np.concatenate([Was, Wad], axis=1)
    WWl_full = np.concatenate(
        [Wf[:, h * D:(h + 1) * D] @ Wlf[h * D:(h + 1) * D, :]
         for h in range(H)], axis=0)
    n_ch = H * D // P
    WWl = np.concatenate([WWl_full[ch * P:(ch + 1) * P, :]
                          for ch in range(n_ch)], axis=1)
    blp = (np.asarray(bias, np.float64) @ Wlf + np.asarray(bl, np.float64))

    src = np.concatenate([np.asarray(edge_index[0]),
                          np.arange(N, dtype=np.int64)]).astype(np.int64)
    dst = np.concatenate([np.asarray(edge_index[1]),
                          np.arange(N, dtype=np.int64)]).astype(np.int64)
    order = np.argsort(dst, kind="stable")
    src = src[order].astype(np.int64)
    dst = dst[order].astype(np.int64)

    # group each block's edges by src window; T_W = max run tiles
    blk = dst >> 7
    win = src >> 15
    key = blk * n_win + win
    order2 = np.argsort(key, kind="stable")
    src, dst, key, win = src[order2], dst[order2], key[order2], win[order2]
    run_counts = np.bincount(key, minlength=BLOCKS * N_CORES * n_win)
    T_W = max(1, int(np.max((run_counts + P - 1) // P)))
    TPB = n_win * T_W
    TILES = BLOCKS * TPB
    run_starts = np.zeros(len(run_counts) + 1, np.int64)
    np.cumsum(run_counts, out=run_starts[1:])

    x_np = np.asarray(x, np.float32)
    x_ext = np.zeros((N_PAD, XW), NP_BF16)
    x_ext[:N, :D] = x_np.astype(NP_BF16)
    x_ext[:N, D] = np.float32(1.0).astype(NP_BF16)
    xT = np.zeros((D, N_PAD), NP_BF16)
    xT[:, :N] = x_np.T.astype(NP_BF16)

    src_cores, dst_cores, dl_cores = [], [], []
    for c in range(N_CORES):
        s16 = np.zeros(TILES * P, np.int64)
        d16 = np.zeros(TILES * P, np.int64)
        dl = np.full((TILES, P), 255.0, np.float32)
        for b in range(BLOCKS):
            gb = c * BLOCKS + b
            for w in range(n_win):
                r = gb * n_win + w
                s0, cnt = run_starts[r], run_counts[r]
                base = (b * TPB + w * T_W) * P
                if cnt:
                    sl = slice(s0, s0 + cnt)
                    s16[base:base + cnt] = src[sl] - w * WIN
                    d16[base:base + cnt] = dst[sl] - c * BLOCKS * P
                    tv = dl[b * TPB + w * T_W: b * TPB + (w + 1) * T_W]
                    fl = tv.reshape(-1)
                    fl[:cnt] = (dst[sl] - gb * P).astype(np.float32)
        src_cores.append((_wrap16(s16), _wrap16(d16),
                          np.ascontiguousarray(
                              xT[:, c * BLOCKS * P:(c + 1) * BLOCKS * P])))
        dst_cores.append(None)
        # dl is [TILES, P] in edge order i = g*128 + p -> [P, TILES]
        dl_cores.append(np.ascontiguousarray(dl.T.astype(NP_BF16)))

    consts = {
        "Wa": Wa.astype(NP_BF16),
        "WWl": WWl.astype(NP_BF16),
        "blp": blp.reshape(1, D).astype(NP_BF16),
        "ident": np.eye(P, dtype=NP_BF16),
        "iota": np.tile(np.arange(P, dtype=np.float32).astype(NP_BF16), (P, 1)),
        "ones_r": np.ones((1, P), NP_BF16),
        "xT": xT,
    }
    for w in range(n_win):
        consts[f"x_ext{w}"] = np.ascontiguousarray(
            x_ext[w * WIN: min((w + 1) * WIN, N_PAD)])
    meta = dict(N=N, D=D, H=H, N_PAD=N_PAD, BLOCKS=BLOCKS, T_W=T_W,
                TPB=TPB, TILES=TILES)
    return consts, src_cores, dst_cores, dl_cores, meta


_PROG_CACHE = {}
LAST_EXEC_NS = None


def _run_pjrt(nc, in_maps, n_cores, bench_iters=0):
    """Execute via PJRT (axon) without output donation; optionally re-run
    for wall-clock timing."""
    import time
    import jax
    from jax.experimental.shard_map import shard_map
    from jax.sharding import Mesh, PartitionSpec
    from concourse import bass2jax, mybir as mb

    bass2jax.install_neuronx_cc_hook()
    partition_name = (nc.partition_id_tensor.name
                      if nc.partition_id_tensor else None)

    in_names, out_names, out_avals, zero_outs = [], [], [], []
    for alloc in nc.m.functions[0].allocations:
        if not isinstance(alloc, mb.MemoryLocationSet):
            continue
        name = alloc.memorylocations[0].name
        if alloc.kind == "ExternalInput":
            if name != partition_name:
                in_names.append(name)
        elif alloc.kind == "ExternalOutput":
            shape = tuple(alloc.tensor_shape)
            dtype = mb.dt.np(alloc.dtype)
            out_names.append(name)
            out_avals.append(jax.core.ShapedArray(shape, dtype))
            zero_outs.append(np.zeros(shape, dtype))
    n_params = len(in_names)
    all_in_names = in_names + out_names + ([partition_name]
                                           if partition_name else [])

    def _body(*args):
        operands = list(args)
        if partition_name is not None:
            operands.append(bass2jax.partition_id_tensor())
        outs = bass2jax._bass_exec_p.bind(
            *operands,
            out_avals=tuple(out_avals),
            in_names=tuple(all_in_names),
            out_names=tuple(out_names),
            lowering_input_output_aliases=(),
            sim_require_finite=True,
            sim_require_nnan=True,
            nc=nc,
        )
        return tuple(outs)

    devices = jax.devices()[:n_cores]
    mesh = Mesh(np.asarray(devices), ("core",))
    n_outs = len(out_names)
    sharded = jax.jit(
        shard_map(_body, mesh=mesh,
                  in_specs=(PartitionSpec("core"),) * (n_params + n_outs),
                  out_specs=(PartitionSpec("core"),) * n_outs,
                  check_rep=False),
        keep_unused=True,
    )
    concat_in = [
        np.concatenate([np.asarray(in_maps[c][nm]) for c in range(n_cores)],
                       axis=0)
        for nm in in_names
    ]
    concat_zeros = [np.zeros((n_cores * z.shape[0], *z.shape[1:]), z.dtype)
                    for z in zero_outs]
    dev_args = [jax.device_put(a) for a in (*concat_in, *concat_zeros)]
    out_arrs = sharded(*dev_args)
    jax.block_until_ready(out_arrs)

    best_ns = None
    if bench_iters:
        times = []
        for _ in range(bench_iters):
            t0 = time.perf_counter_ns()
            r = sharded(*dev_args)
            jax.block_until_ready(r)
            times.append(time.perf_counter_ns() - t0)
        best_ns = min(times)
        print(f"[bench] wall ns per launch: min={min(times)} "
              f"med={sorted(times)[len(times)//2]} max={max(times)}",
              flush=True)

    results = [
        {nm: np.asarray(out_arrs[i]).reshape(n_cores, *out_avals[i].shape)[c]
         for i, nm in enumerate(out_names)}
        for c in range(n_cores)
    ]
    return results, best_ns


def kernel(x, edge_index, W, att_src, att_dst, bias, Wl, bl):
    global LAST_EXEC_NS
    consts, src_cores, dst_cores, dl_cores, meta = _host_prep(
        x, edge_index, W, att_src, att_dst, bias, Wl, bl)
    N, D, H = meta["N"], meta["D"], meta["H"]

    key = (meta["N_PAD"], meta["BLOCKS"], meta["T_W"], D, H)
    if key not in _PROG_CACHE:
        _PROG_CACHE[key] = build_program(meta["N_PAD"], meta["BLOCKS"],
                                         meta["T_W"], D, H)
    nc = _PROG_CACHE[key]

    in_maps = []
    for c in range(N_CORES):
        m = dict(consts)
        m["src16"], m["dst16"], m["xTd"] = src_cores[c]
        m["dstloc"] = dl_cores[c]
        in_maps.append(m)

    if os.environ.get("BASS_GAT_SIM"):
        from concourse.bass_interp import CoreSim
        outs = []
        for c in range(N_CORES):
            sim = CoreSim(nc)
            for k, v in in_maps[c].items():
                sim.tensor(k)[:] = v
            sim.simulate()
            outs.append(np.array(sim.tensor("out")))
    else:
        bench = int(os.environ.get("BASS_GAT_BENCH", "0"))
        results, best_ns = _run_pjrt(nc, in_maps, N_CORES, bench_iters=bench)
        outs = [r["out"] for r in results]
        LAST_EXEC_NS = best_ns
    full = np.concatenate(outs, axis=0)[:N]
    return np.ascontiguousarray(full.astype(np.float32))



# revision 5
# speedup vs baseline: 1.4643x; 1.4643x over previous
"""GAT multi-head block on 8 Trainium2 NeuronCores — v3.

Edge-parallel, dst-sharded. Host sorts the E+N edges (self-loops added) by
dst, shards dst blocks of 128 nodes across cores (98 blocks/core), groups
each block's edges by src int16-window (dma_gather index limit), pads each
(block, window) run to T_W tiles of 128 slots. Fused host weights:
  Wa = [W_h@att_src_h | W_h@att_dst_h] (64x8), WWl_h = W_h@Wl_h,
  blp = bias@Wl + bl.
Device phase A: A = x @ Wa; a_src written into x_ext rows (cols 66:70),
a_dst for the core's own nodes into A_loc row heads (cols 0:4).
Device phase B per block: one dma_gather per window for x_ext rows (256B,
[x|1|asrc]) + one dma_gather for A_loc rows by dst, all tiles per call;
u = exp(leakyrelu(asrc+adst)) block-batched; one-hot eq[e,v]=(dst_local==v)
block-batched; rhs = concat_h(u_h*[x|1]) block-batched (4D APs); one PE
matmul per tile accumulates per-head weighted x-sums + softmax denominators
in PSUM. Per block: normalize, transpose, fused WWl_h + bias, write out.
Inputs are device_put with NamedSharding (no per-launch resharding).
"""

import os
import sys
import numpy as np

for _p in ("/opt/trn_rl_repo",):
    if _p not in sys.path:
        sys.path.insert(0, _p)

import concourse.bass as bass
import concourse.bacc as bacc
import concourse.mybir as mybir
import concourse.tile as tile

F32 = mybir.dt.float32
BF16 = mybir.dt.bfloat16
I16 = mybir.dt.int16
NP_BF16 = np.dtype(mybir.dt.np(BF16))

NEG_SLOPE = 0.2
P = 128
N_CORES = 8
D = 64
H = 4
N = 100000
NBLK_TOTAL = (N + P - 1) // P                     # 782
BLOCKS = (NBLK_TOTAL + N_CORES - 1) // N_CORES    # 98
N_PAD = BLOCKS * N_CORES * P                      # 100352
NT = N_PAD // P                                   # 784
NLOC = BLOCKS * P                                 # 12544
WIN = 32768
N_WIN = (N_PAD + WIN - 1) // WIN                  # 4
XW = 128                                          # 256B gather rows
ONE_COL = D                                       # 64
ASRC_COL = 66
RW = H * (D + 1)                                  # 260
N_CH = H * D // P                                 # 2


def _ap(t, offset_elems, dims):
    return bass.AP(t, offset_elems, [list(d) for d in dims])


def build_program(T_W):
    TPB = N_WIN * T_W

    nc = bacc.Bacc("TRN2", target_bir_lowering=False, debug=False,
                   num_devices=N_CORES)

    x_ext = nc.declare_dram_parameter("x_ext", [N_PAD, XW], BF16,
                                      isOutput=False)
    xT = nc.declare_dram_parameter("xT", [D, N_PAD], BF16, isOutput=False)
    xTd = nc.declare_dram_parameter("xTd", [D, NLOC], BF16, isOutput=False)
    # consts [128, 456]: ident | iota | Wa(rows 0:64) | WWl | blp(row 0)
    C_IDENT, C_IOTA, C_WA, C_WWL, C_BLP = 0, 128, 256, 264, 392
    consts = nc.declare_dram_parameter("consts", [P, 456], BF16,
                                       isOutput=False)
    src16 = nc.declare_dram_parameter("src16", [BLOCKS * P, TPB * 8], I16,
                                      isOutput=False)
    dst16 = nc.declare_dram_parameter("dst16", [BLOCKS * P, TPB * 8], I16,
                                      isOutput=False)
    dloc = nc.declare_dram_parameter("dloc", [BLOCKS * P, TPB], BF16,
                                     isOutput=False)
    out = nc.declare_dram_parameter("out", [NLOC, D], F32, isOutput=True)
    A_loc = nc.declare_dram_parameter("A_loc", [NLOC, XW], BF16,
                                      isOutput=False)

    A_SLAB = 64
    n_slabs = (NT + A_SLAB - 1) // A_SLAB

    with tile.TileContext(nc) as tc:
        with tc.tile_pool(name="const", bufs=1) as cpool:
            c_sb = cpool.tile([P, 456], BF16, tag="consts")
            nc.sync.dma_start(out=c_sb[:], in_=consts[:])
            ones_sb = cpool.tile([1, P], BF16, tag="ones")
            nc.vector.memset(ones_sb[:], 1.0)
            ident_sb = c_sb[:, C_IDENT:C_IDENT + P]
            wa_sb = c_sb[0:D, C_WA:C_WA + 2 * H]
            wwl_sb = c_sb[:, C_WWL:C_WWL + N_CH * D]
            blp_sb = c_sb[0:1, C_BLP:C_BLP + D]

            # ---------- phase A: [a_src | a_dst] = x @ Wa ----------
            with (
                tc.tile_pool(name="a_xt", bufs=2) as xt_pool,
                tc.tile_pool(name="a_ps", bufs=4, space="PSUM") as aps_pool,
                tc.tile_pool(name="a_st", bufs=2) as ast_pool,
            ):
                # A1: a_src for all nodes -> x_ext cols 66:70
                for s in range(n_slabs):
                    t0 = s * A_SLAB
                    nt = min(A_SLAB, NT - t0)
                    slab = xt_pool.tile([D, A_SLAB * P], BF16, tag="slab")
                    nc.sync.dma_start(out=slab[:, : nt * P],
                                      in_=xT[:, t0 * P:(t0 + nt) * P])
                    stage = ast_pool.tile([P, A_SLAB, 2 * H], BF16, tag="ast")
                    for t in range(nt):
                        aps = aps_pool.tile([P, 2 * H], F32, space="PSUM",
                                            tag="aps")
                        nc.tensor.matmul(aps[:], slab[:, t * P:(t + 1) * P],
                                         wa_sb, start=True, stop=True)
                        nc.any.tensor_copy(out=stage[:, t, :], in_=aps[:])
                    nc.sync.dma_start(
                        out=_ap(x_ext, t0 * P * XW + ASRC_COL,
                                [[XW, P], [P * XW, nt], [1, H]]),
                        in_=stage[:, :nt, 0:H])
                # A2: a_dst for this core's own nodes -> A_loc cols 0:4
                NTd = NLOC // P
                n_slabs_d = (NTd + A_SLAB - 1) // A_SLAB
                for s in range(n_slabs_d):
                    t0 = s * A_SLAB
                    nt = min(A_SLAB, NTd - t0)
                    slab = xt_pool.tile([D, A_SLAB * P], BF16, tag="slab")
                    nc.sync.dma_start(out=slab[:, : nt * P],
                                      in_=xTd[:, t0 * P:(t0 + nt) * P])
                    stage = ast_pool.tile([P, A_SLAB, 2 * H], BF16, tag="ast")
                    for t in range(nt):
                        aps = aps_pool.tile([P, 2 * H], F32, space="PSUM",
                                            tag="aps")
                        nc.tensor.matmul(aps[:], slab[:, t * P:(t + 1) * P],
                                         wa_sb, start=True, stop=True)
                        nc.any.tensor_copy(out=stage[:, t, :], in_=aps[:])
                    nc.sync.dma_start(
                        out=_ap(A_loc, t0 * P * XW,
                                [[XW, P], [P * XW, nt], [1, H]]),
                        in_=stage[:, :nt, H:2 * H])

            # ---------- phase B ----------
            with (
                tc.tile_pool(name="idx", bufs=3) as idx_pool,
                tc.tile_pool(name="gx", bufs=3) as gx_pool,
                tc.tile_pool(name="ag", bufs=3) as ag_pool,
                tc.tile_pool(name="uexp", bufs=2) as u_pool,
                tc.tile_pool(name="eq", bufs=2) as eq_pool,
                tc.tile_pool(name="rhs", bufs=2) as rhs_pool,
                tc.tile_pool(name="m1", bufs=2, space="PSUM") as m1_pool,
                tc.tile_pool(name="post_ps", bufs=2, space="PSUM") as pps_pool,
                tc.tile_pool(name="post_sb", bufs=3) as psb_pool,
                tc.tile_pool(name="fout", bufs=2) as fout_pool,
            ):
                for b in range(BLOCKS):
                    s_sb = idx_pool.tile([P, TPB * 8], I16, tag="s16")
                    nc.sync.dma_start(out=s_sb[:],
                                      in_=src16[b * P:(b + 1) * P, :])
                    d_sb = idx_pool.tile([P, TPB * 8], I16, tag="d16")
                    nc.sync.dma_start(out=d_sb[:],
                                      in_=dst16[b * P:(b + 1) * P, :])
                    dl_sb = idx_pool.tile([P, TPB], BF16, tag="dl")
                    nc.sync.dma_start(out=dl_sb[:],
                                      in_=dloc[b * P:(b + 1) * P, :])

                    gxb = gx_pool.tile([P, TPB, XW], BF16, tag="gx")
                    for w in range(N_WIN):
                        rows_w = min(WIN, N_PAD - w * WIN)
                        nc.gpsimd.dma_gather(
                            gxb[:, w * T_W:(w + 1) * T_W, :],
                            _ap(x_ext, w * WIN * XW,
                                [[XW, rows_w], [1, XW]]),
                            s_sb[:, w * T_W * 8:(w + 1) * T_W * 8],
                            T_W * P, T_W * P, XW, single_packet=False)
                    agD = ag_pool.tile([P, TPB, XW], BF16, tag="agD")
                    nc.gpsimd.dma_gather(
                        agD[:], A_loc[:], d_sb[:],
                        TPB * P, TPB * P, XW, single_packet=False)

                    # u = exp(leaky_relu(asrc + adst)) for the whole block
                    lg = u_pool.tile([P, TPB, H], F32, tag="lg")
                    nc.vector.tensor_add(
                        out=lg[:],
                        in0=_ap(gxb.tensor, gxb.offset + ASRC_COL,
                                [list(gxb.ap[0]), [XW, TPB], [1, H]]),
                        in1=_ap(agD.tensor, agD.offset,
                                [list(agD.ap[0]), [XW, TPB], [1, H]]))
                    lr = u_pool.tile([P, TPB, H], F32, tag="lr")
                    nc.vector.scalar_tensor_tensor(
                        out=lr[:], in0=lg[:], scalar=NEG_SLOPE, in1=lg[:],
                        op0=mybir.AluOpType.mult, op1=mybir.AluOpType.max)
                    ue = u_pool.tile([P, TPB, H], BF16, tag="ue")
                    nc.scalar.activation(out=ue[:], in_=lr[:],
                                         func=mybir.ActivationFunctionType.Exp)

                    # eq[p, t, v] = (dl[p, t] == v), all tiles at once
                    eqb = eq_pool.tile([P, TPB, P], BF16, tag="eqb")
                    nc.vector.tensor_tensor(
                        out=eqb[:],
                        in0=_ap(dl_sb.tensor, dl_sb.offset,
                                [list(dl_sb.ap[0]), [1, TPB], [0, P]]),
                        in1=_ap(c_sb.tensor, c_sb.offset + C_IOTA,
                                [list(c_sb.ap[0]), [0, TPB], [1, P]]),
                        op=mybir.AluOpType.is_equal)
                    # rhs[p, t, h, c] = gx[p, t, c] * ue[p, t, h], c in 0..64
                    rhs = rhs_pool.tile([P, TPB, RW], BF16, tag="rhs")
                    nc.vector.tensor_mul(
                        out=_ap(rhs.tensor, rhs.offset,
                                [list(rhs.ap[0]), [RW, TPB],
                                 [D + 1, H], [1, D + 1]]),
                        in0=_ap(gxb.tensor, gxb.offset,
                                [list(gxb.ap[0]), [XW, TPB],
                                 [0, H], [1, D + 1]]),
                        in1=_ap(ue.tensor, ue.offset,
                                [list(ue.ap[0]), [H, TPB],
                                 [1, H], [0, D + 1]]))

                    m1_ps = m1_pool.tile([P, RW], F32, space="PSUM", tag="m1")
                    for t in range(TPB):
                        nc.tensor.matmul(
                            m1_ps[:],
                            _ap(eqb.tensor, eqb.offset + t * P,
                                [list(eqb.ap[0]), [1, P]]),
                            _ap(rhs.tensor, rhs.offset + t * RW,
                                [list(rhs.ap[0]), [1, RW]]),
                            start=(t == 0), stop=(t == TPB - 1))

                    # ---- block post ----
                    m1_t = m1_ps.tensor
                    rcp = psb_pool.tile([P, H], F32, tag="rcp")
                    nc.vector.tensor_scalar_add(
                        out=rcp[:],
                        in0=_ap(m1_t, m1_ps.offset + D,
                                [list(m1_ps.ap[0]), [D + 1, H]]),
                        scalar1=1e-16)
                    nc.vector.reciprocal(out=rcp[:], in_=rcp[:])
                    m1n = psb_pool.tile([P, H * D], BF16, tag="m1n")
                    nc.vector.tensor_mul(
                        out=_ap(m1n.tensor, m1n.offset,
                                [list(m1n.ap[0]), [D, H], [1, D]]),
                        in0=_ap(m1_t, m1_ps.offset,
                                [list(m1_ps.ap[0]), [D + 1, H], [1, D]]),
                        in1=_ap(rcp.tensor, rcp.offset,
                                [list(rcp.ap[0]), [1, H], [0, D]]))
                    f_ps = pps_pool.tile([P, D], F32, space="PSUM", tag="fps")
                    for ch in range(N_CH):
                        tp = pps_pool.tile([P, P], BF16, space="PSUM",
                                           tag="tp")
                        nc.tensor.transpose(
                            tp[:], m1n[:, ch * P:(ch + 1) * P], ident_sb)
                        tps = psb_pool.tile([P, P], BF16, tag="tps")
                        nc.any.tensor_copy(out=tps[:], in_=tp[:])
                        nc.tensor.matmul(f_ps[:], tps[:],
                                         wwl_sb[:, ch * D:(ch + 1) * D],
                                         start=(ch == 0), stop=False)
                    nc.tensor.matmul(f_ps[:], ones_sb[:], blp_sb,
                                     start=False, stop=True)
                    f_sb = fout_pool.tile([P, D], F32, tag="fsb")
                    nc.any.tensor_copy(out=f_sb[:], in_=f_ps[:])
                    nc.sync.dma_start(out=out[b * P:(b + 1) * P, :],
                                      in_=f_sb[:])

    nc.compile()
    return nc


def _host_prep(x, edge_index, W, att_src, att_dst, bias, Wl, bl):
    # fused weights (float64 for clean folding)
    Wf = np.asarray(W, np.float64)
    Wlf = np.asarray(Wl, np.float64)
    Was = np.stack([Wf[:, h * D:(h + 1) * D]
                    @ np.asarray(att_src[h], np.float64)
                    for h in range(H)], axis=1)
    Wad = np.stack([Wf[:, h * D:(h + 1) * D]
                    @ np.asarray(att_dst[h], np.float64)
                    for h in range(H)], axis=1)
    Wa = np.concatenate([Was, Wad], axis=1)               # [64, 8]
    WWl_full = np.concatenate(
        [Wf[:, h * D:(h + 1) * D] @ Wlf[h * D:(h + 1) * D, :]
         for h in range(H)], axis=0)                      # [256, 64]
    WWl = np.concatenate([WWl_full[ch * P:(ch + 1) * P, :]
                          for ch in range(N_CH)], axis=1)  # [128, 128]
    blp = (np.asarray(bias, np.float64) @ Wlf
           + np.asarray(bl, np.float64))                  # [64]

    consts = np.zeros((P, 456), NP_BF16)
    consts[:, 0:P] = np.eye(P, dtype=NP_BF16)
    consts[:, P:2 * P] = np.tile(
        np.arange(P, dtype=np.float32).astype(NP_BF16), (P, 1))
    consts[0:D, 256:264] = Wa.astype(NP_BF16)
    consts[:, 264:392] = WWl.astype(NP_BF16)
    consts[0:1, 392:456] = blp.reshape(1, D).astype(NP_BF16)

    # edge tables: sort by dst, then group each block's edges by src window
    src = np.concatenate([np.asarray(edge_index[0]),
                          np.arange(N, dtype=np.int64)]).astype(np.int64)
    dst = np.concatenate([np.asarray(edge_index[1]),
                          np.arange(N, dtype=np.int64)]).astype(np.int64)
    order = np.argsort(dst, kind="stable")
    src = src[order]
    dst = dst[order]
    blk = dst >> 7
    win = src >> 15
    key = blk * N_WIN + win
    order2 = np.argsort(key, kind="stable")
    src, dst, key, win, blk = (src[order2], dst[order2], key[order2],
                               win[order2], blk[order2])
    run_cnt = np.bincount(key, minlength=NT * N_WIN)
    T_W = max(1, int(np.max((run_cnt + P - 1) // P)))
    TPB = N_WIN * T_W
    run_starts = np.zeros(len(run_cnt) + 1, np.int64)
    np.cumsum(run_cnt, out=run_starts[1:])

    jr = np.arange(len(dst), dtype=np.int64) - run_starts[key]
    t_loc = win * T_W + jr // P                    # tile within block
    p = jr % P
    core = (blk // BLOCKS).astype(np.int64)
    b_loc = (blk % BLOCKS).astype(np.int64)

    # flat slot i = t*128 + p within each block's TPB*128 slots
    sv = np.zeros((N_CORES, BLOCKS, TPB * P), np.int64)
    dv = np.zeros((N_CORES, BLOCKS, TPB * P), np.int64)
    sv[core, b_loc, t_loc * P + p] = src - win * WIN
    dv[core, b_loc, t_loc * P + p] = dst - core * NLOC
    dl8 = np.full((N_CORES, BLOCKS * P, TPB), 255.0, np.float32)
    dl8[core, b_loc * P + p, t_loc] = (dst & 127).astype(np.float32)
    dl8 = dl8.astype(NP_BF16)

    def wrap16(v):     # [C, B, TPB*128] -> [C, B*128, TPB*8]
        a = v.reshape(N_CORES, BLOCKS, TPB, 8, 16).astype(np.int16)
        a = a.transpose(0, 1, 4, 2, 3).reshape(N_CORES, BLOCKS, 16, TPB * 8)
        a = np.tile(a, (1, 1, 8, 1))
        return a.reshape(N_CORES, BLOCKS * P, TPB * 8)

    src16 = wrap16(sv)
    dst16 = wrap16(dv)

    x_np = np.asarray(x, np.float32)
    x_ext = np.zeros((N_PAD, XW), NP_BF16)
    x_ext[:N, :D] = x_np.astype(NP_BF16)
    x_ext[:, ONE_COL] = np.float32(1.0).astype(NP_BF16)
    xT = np.zeros((D, N_PAD), NP_BF16)
    xT[:, :N] = x_np.T.astype(NP_BF16)

    shared = {"x_ext": x_ext, "xT": xT, "consts": consts,
              "A_loc": np.zeros((NLOC, XW), NP_BF16)}
    percore = []
    for c in range(N_CORES):
        percore.append({
            "src16": src16[c], "dst16": dst16[c], "dloc": dl8[c],
            "xTd": np.ascontiguousarray(xT[:, c * NLOC:(c + 1) * NLOC]),
        })
    return shared, percore, T_W


_PROG_CACHE = {}
LAST_EXEC_NS = None


def _run_pjrt(nc, in_maps, n_cores, bench_iters=0):
    """Execute via PJRT (axon) with pre-sharded device buffers; optionally
    re-run for wall-clock timing."""
    import time
    import jax
    from jax.experimental.shard_map import shard_map
    from jax.sharding import Mesh, PartitionSpec, NamedSharding
    from concourse import bass2jax, mybir as mb

    bass2jax.install_neuronx_cc_hook()
    partition_name = (nc.partition_id_tensor.name
                      if nc.partition_id_tensor else None)

    in_names, out_names, out_avals, zero_outs = [], [], [], []
    for alloc in nc.m.functions[0].allocations:
        if not isinstance(alloc, mb.MemoryLocationSet):
            continue
        name = alloc.memorylocations[0].name
        if alloc.kind == "ExternalInput":
            if name != partition_name:
                in_names.append(name)
        elif alloc.kind == "ExternalOutput":
            shape = tuple(alloc.tensor_shape)
            dtype = mb.dt.np(alloc.dtype)
            out_names.append(name)
            out_avals.append(jax.core.ShapedArray(shape, dtype))
            zero_outs.append(np.zeros(shape, dtype))
    n_params = len(in_names)
    all_in_names = in_names + out_names + ([partition_name]
                                           if partition_name else [])

    def _body(*args):
        operands = list(args)
        if partition_name is not None:
            operands.append(bass2jax.partition_id_tensor())
        outs = bass2jax._bass_exec_p.bind(
            *operands,
            out_avals=tuple(out_avals),
            in_names=tuple(all_in_names),
            out_names=tuple(out_names),
            lowering_input_output_aliases=(),
            sim_require_finite=True,
            sim_require_nnan=True,
            nc=nc,
        )
        return tuple(outs)

    devices = jax.devices()[:n_cores]
    mesh = Mesh(np.asarray(devices), ("core",))
    n_outs = len(out_names)
    sharded = jax.jit(
        shard_map(_body, mesh=mesh,
                  in_specs=(PartitionSpec("core"),) * (n_params + n_outs),
                  out_specs=(PartitionSpec("core"),) * n_outs,
                  check_rep=False),
        keep_unused=True,
    )
    concat_in = [
        np.concatenate([np.asarray(in_maps[c][nm]) for c in range(n_cores)],
                       axis=0)
        for nm in in_names
    ]
    concat_zeros = [np.zeros((n_cores * z.shape[0], *z.shape[1:]), z.dtype)
                    for z in zero_outs]
    shard = NamedSharding(mesh, PartitionSpec("core"))
    dev_args = [jax.device_put(a, shard)
                for a in (*concat_in, *concat_zeros)]
    out_arrs = sharded(*dev_args)
    jax.block_until_ready(out_arrs)

    best_ns = None
    if bench_iters:
        times = []
        for _ in range(bench_iters):
            t0 = time.perf_counter_ns()
            r = sharded(*dev_args)
            jax.block_until_ready(r)
            times.append(time.perf_counter_ns() - t0)
        best_ns = min(times)
        print(f"[bench] wall ns per launch: min={min(times)} "
              f"med={sorted(times)[len(times)//2]} max={max(times)}",
              flush=True)

    results = [
        {nm: np.asarray(out_arrs[i]).reshape(n_cores, *out_avals[i].shape)[c]
         for i, nm in enumerate(out_names)}
        for c in range(n_cores)
    ]
    return results, best_ns


def kernel(x, edge_index, W, att_src, att_dst, bias, Wl, bl):
    global LAST_EXEC_NS
    shared, percore, T_W = _host_prep(
        x, edge_index, W, att_src, att_dst, bias, Wl, bl)

    if T_W not in _PROG_CACHE:
        _PROG_CACHE[T_W] = build_program(T_W)
    nc = _PROG_CACHE[T_W]

    in_maps = [dict(shared, **percore[c]) for c in range(N_CORES)]

    if os.environ.get("BASS_GAT_SIM"):
        from concourse.bass_interp import CoreSim
        outs = []
        for c in range(int(os.environ.get("BASS_GAT_SIM_CORES", N_CORES))):
            sim = CoreSim(nc)
            for k, v in in_maps[c].items():
                sim.tensor(k)[:] = v
            sim.simulate()
            outs.append(np.array(sim.tensor("out")))
        while len(outs) < N_CORES:
            outs.append(np.zeros((NLOC, D), np.float32))
    else:
        bench = int(os.environ.get("BASS_GAT_BENCH", "5"))
        results, best_ns = _run_pjrt(nc, in_maps, N_CORES, bench_iters=bench)
        outs = [r["out"] for r in results]
        LAST_EXEC_NS = best_ns
    full = np.concatenate(outs, axis=0)[:N]
    return np.ascontiguousarray(full.astype(np.float32))


# revision 14
# speedup vs baseline: 1.5129x; 1.0332x over previous
"""GAT multi-head block on 8 Trainium2 NeuronCores — v3.

Edge-parallel, dst-sharded. Host sorts the E+N edges (self-loops added) by
dst, shards dst blocks of 128 nodes across cores (98 blocks/core), groups
each block's edges by src int16-window (dma_gather index limit), pads each
(block, window) run to T_W tiles of 128 slots. Fused host weights:
  Wa = [W_h@att_src_h | W_h@att_dst_h] (64x8), WWl_h = W_h@Wl_h,
  blp = bias@Wl + bl.
Device phase A: A = x @ Wa; a_src written into x_ext rows (cols 66:70),
a_dst for the core's own nodes into A_loc [NLOC, 4].
Device phase B per block: one dma_gather per window for x_ext rows (256B,
[x|1|asrc]); one-hot eq[e,v]=(dst_local==v) block-batched; per-edge a_dst
computed on the PE (per tile: transpose eq -> eqT, matmul eqT @ a_dst_blk
into PSUM) — no per-edge a_dst gather, halving SWDGE descriptor-gen which
is the ~7.6ns/row bottleneck; u = exp(leakyrelu(asrc+adst)) block-batched;
rhs = concat_h(u_h*[x|1]) block-batched (4D APs); one PE matmul per tile
accumulates per-head weighted x-sums + softmax denominators in PSUM. Per
block: normalize, transpose, fused WWl_h + bias, write out. Inputs are
device_put with NamedSharding (no per-launch resharding).
"""

import os
import sys
import numpy as np

for _p in ("/opt/trn_rl_repo",):
    if _p not in sys.path:
        sys.path.insert(0, _p)

import concourse.bass as bass
import concourse.bacc as bacc
import concourse.mybir as mybir
import concourse.tile as tile

F32 = mybir.dt.float32
BF16 = mybir.dt.bfloat16
I16 = mybir.dt.int16
NP_BF16 = np.dtype(mybir.dt.np(BF16))

NEG_SLOPE = 0.2
P = 128
N_CORES = 8
D = 64
H = 4
N = 100000
NBLK_TOTAL = (N + P - 1) // P                     # 782
BLOCKS = (NBLK_TOTAL + N_CORES - 1) // N_CORES    # 98
N_PAD = BLOCKS * N_CORES * P                      # 100352
NT = N_PAD // P                                   # 784
NLOC = BLOCKS * P                                 # 12544
WIN = 32768
N_WIN = (N_PAD + WIN - 1) // WIN                  # 4
XW = 128                                          # 256B gather rows
ONE_COL = D                                       # 64
ASRC_COL = 66
RW = H * (D + 1)                                  # 260
N_CH = H * D // P                                 # 2


def _ap(t, offset_elems, dims):
    return bass.AP(t, offset_elems, [list(d) for d in dims])


def build_program(T_W):
    TPB = N_WIN * T_W

    nc = bacc.Bacc("TRN2", target_bir_lowering=False, debug=False,
                   num_devices=N_CORES)

    x_ext = nc.declare_dram_parameter("x_ext", [N_PAD, XW], BF16,
                                      isOutput=False)
    xT = nc.declare_dram_parameter("xT", [D, N_PAD], BF16, isOutput=False)
    xTd = nc.declare_dram_parameter("xTd", [D, NLOC], BF16, isOutput=False)
    # consts [128, 456]: ident | iota | Wa(rows 0:64) | WWl | blp(row 0)
    C_IDENT, C_IOTA, C_WA, C_WWL, C_BLP = 0, 128, 256, 264, 392
    consts = nc.declare_dram_parameter("consts", [P, 456], BF16,
                                       isOutput=False)
    src16 = nc.declare_dram_parameter("src16", [BLOCKS * P, TPB * 8], I16,
                                      isOutput=False)
    dloc = nc.declare_dram_parameter("dloc", [BLOCKS * P, TPB], BF16,
                                     isOutput=False)
    out = nc.declare_dram_parameter("out", [NLOC, D], F32, isOutput=True)
    A_loc = nc.dram_tensor("A_loc", [NLOC, H], BF16)

    A_SLAB = 64
    n_slabs = (NT + A_SLAB - 1) // A_SLAB

    with tile.TileContext(nc) as tc:
        with tc.tile_pool(name="const", bufs=1) as cpool:
            c_sb = cpool.tile([P, 456], BF16, tag="consts")
            nc.sync.dma_start(out=c_sb[:], in_=consts[:])
            ones_sb = cpool.tile([1, P], BF16, tag="ones")
            nc.vector.memset(ones_sb[:], 1.0)
            ident_sb = c_sb[:, C_IDENT:C_IDENT + P]
            wa_sb = c_sb[0:D, C_WA:C_WA + 2 * H]
            wwl_sb = c_sb[:, C_WWL:C_WWL + N_CH * D]
            blp_sb = c_sb[0:1, C_BLP:C_BLP + D]

            # ---------- phase A: [a_src | a_dst] = x @ Wa ----------
            with (
                tc.tile_pool(name="a_xt", bufs=2) as xt_pool,
                tc.tile_pool(name="a_ps", bufs=4, space="PSUM") as aps_pool,
                tc.tile_pool(name="a_st", bufs=2) as ast_pool,
            ):
                # A1: a_src for all nodes -> x_ext cols 66:70
                for s in range(n_slabs):
                    t0 = s * A_SLAB
                    nt = min(A_SLAB, NT - t0)
                    slab = xt_pool.tile([D, A_SLAB * P], BF16, tag="slab")
                    nc.sync.dma_start(out=slab[:, : nt * P],
                                      in_=xT[:, t0 * P:(t0 + nt) * P])
                    stage = ast_pool.tile([P, A_SLAB, 2 * H], BF16, tag="ast")
                    for t in range(nt):
                        aps = aps_pool.tile([P, 2 * H], F32, space="PSUM",
                                            tag="aps")
                        nc.tensor.matmul(aps[:], slab[:, t * P:(t + 1) * P],
                                         wa_sb, start=True, stop=True)
                        nc.any.tensor_copy(out=stage[:, t, :], in_=aps[:])
                    nc.sync.dma_start(
                        out=_ap(x_ext, t0 * P * XW + ASRC_COL,
                                [[XW, P], [P * XW, nt], [1, H]]),
                        in_=stage[:, :nt, 0:H])
                # A2: a_dst for this core's own nodes -> A_loc cols 0:4
                NTd = NLOC // P
                n_slabs_d = (NTd + A_SLAB - 1) // A_SLAB
                for s in range(n_slabs_d):
                    t0 = s * A_SLAB
                    nt = min(A_SLAB, NTd - t0)
                    slab = xt_pool.tile([D, A_SLAB * P], BF16, tag="slab")
                    nc.sync.dma_start(out=slab[:, : nt * P],
                                      in_=xTd[:, t0 * P:(t0 + nt) * P])
                    stage = ast_pool.tile([P, A_SLAB, 2 * H], BF16, tag="ast")
                    for t in range(nt):
                        aps = aps_pool.tile([P, 2 * H], F32, space="PSUM",
                                            tag="aps")
                        nc.tensor.matmul(aps[:], slab[:, t * P:(t + 1) * P],
                                         wa_sb, start=True, stop=True)
                        nc.any.tensor_copy(out=stage[:, t, :], in_=aps[:])
                    nc.sync.dma_start(
                        out=_ap(A_loc, t0 * P * H,
                                [[H, P], [P * H, nt], [1, H]]),
                        in_=stage[:, :nt, H:2 * H])

            # ---------- phase B ----------
            with (
                tc.tile_pool(name="idx", bufs=3) as idx_pool,
                tc.tile_pool(name="gx", bufs=3) as gx_pool,
                tc.tile_pool(name="adb", bufs=3) as adb_pool,
                tc.tile_pool(name="uexp", bufs=2) as u_pool,
                tc.tile_pool(name="eq", bufs=2) as eq_pool,
                tc.tile_pool(name="eqt", bufs=4) as eqt_pool,
                tc.tile_pool(name="eqt_ps", bufs=2, space="PSUM") as etp_pool,
                tc.tile_pool(name="ad_ps", bufs=2, space="PSUM") as adp_pool,
                tc.tile_pool(name="rhs", bufs=2) as rhs_pool,
                tc.tile_pool(name="m1", bufs=2, space="PSUM") as m1_pool,
                tc.tile_pool(name="post_ps", bufs=1, space="PSUM") as pps_pool,
                tc.tile_pool(name="post_sb", bufs=3) as psb_pool,
                tc.tile_pool(name="fout", bufs=2) as fout_pool,
            ):
                for b in range(BLOCKS):
                    s_sb = idx_pool.tile([P, TPB * 8], I16, tag="s16")
                    nc.sync.dma_start(out=s_sb[:],
                                      in_=src16[b * P:(b + 1) * P, :])
                    dl_sb = idx_pool.tile([P, TPB], BF16, tag="dl")
                    nc.sync.dma_start(out=dl_sb[:],
                                      in_=dloc[b * P:(b + 1) * P, :])
                    adb = adb_pool.tile([P, H], BF16, tag="adb")
                    nc.sync.dma_start(out=adb[:],
                                      in_=A_loc[b * P:(b + 1) * P, :])

                    gxb = gx_pool.tile([P, TPB, XW], BF16, tag="gx")
                    for w in range(N_WIN):
                        rows_w = min(WIN, N_PAD - w * WIN)
                        nc.gpsimd.dma_gather(
                            gxb[:, w * T_W:(w + 1) * T_W, :],
                            _ap(x_ext, w * WIN * XW,
                                [[XW, rows_w], [1, XW]]),
                            s_sb[:, w * T_W * 8:(w + 1) * T_W * 8],
                            T_W * P, T_W * P, XW, single_packet=False)

                    # eq[p, t, v] = (dl[p, t] == v), all tiles at once
                    eqb = eq_pool.tile([P, TPB, P], BF16, tag="eqb")
                    nc.vector.tensor_tensor(
                        out=eqb[:],
                        in0=_ap(dl_sb.tensor, dl_sb.offset,
                                [list(dl_sb.ap[0]), [1, TPB], [0, P]]),
                        in1=_ap(c_sb.tensor, c_sb.offset + C_IOTA,
                                [list(c_sb.ap[0]), [0, TPB], [1, P]]),
                        op=mybir.AluOpType.is_equal)

                    # per-edge a_dst on PE: adst[e, h] = sum_v eqT[v,e] adb[v,h]
                    ad_ps = adp_pool.tile([P, TPB, H], F32, space="PSUM",
                                          tag="adps")
                    for t in range(TPB):
                        etp = etp_pool.tile([P, P], BF16, space="PSUM",
                                            tag="etp")
                        nc.tensor.transpose(
                            etp[:],
                            _ap(eqb.tensor, eqb.offset + t * P,
                                [list(eqb.ap[0]), [1, P]]),
                            ident_sb)
                        eqt = eqt_pool.tile([P, P], BF16, tag="eqt")
                        nc.any.tensor_copy(out=eqt[:], in_=etp[:])
                        nc.tensor.matmul(ad_ps[:, t, :], eqt[:], adb[:],
                                         start=True, stop=True)

                    # u = exp(leaky_relu(asrc + adst)) for the whole block
                    lg = u_pool.tile([P, TPB, H], F32, tag="lg")
                    nc.vector.tensor_add(
                        out=lg[:],
                        in0=_ap(gxb.tensor, gxb.offset + ASRC_COL,
                                [list(gxb.ap[0]), [XW, TPB], [1, H]]),
                        in1=ad_ps[:])
                    lr = u_pool.tile([P, TPB, H], F32, tag="lr")
                    nc.vector.scalar_tensor_tensor(
                        out=lr[:], in0=lg[:], scalar=NEG_SLOPE, in1=lg[:],
                        op0=mybir.AluOpType.mult, op1=mybir.AluOpType.max)
                    ue = u_pool.tile([P, TPB, H], BF16, tag="ue")
                    nc.scalar.activation(out=ue[:], in_=lr[:],
                                         func=mybir.ActivationFunctionType.Exp)
                    # rhs[p, t, h, c] = gx[p, t, c] * ue[p, t, h], c in 0..64
                    rhs = rhs_pool.tile([P, TPB, RW], BF16, tag="rhs")
                    nc.vector.tensor_mul(
                        out=_ap(rhs.tensor, rhs.offset,
                                [list(rhs.ap[0]), [RW, TPB],
                                 [D + 1, H], [1, D + 1]]),
                        in0=_ap(gxb.tensor, gxb.offset,
                                [list(gxb.ap[0]), [XW, TPB],
                                 [0, H], [1, D + 1]]),
                        in1=_ap(ue.tensor, ue.offset,
                                [list(ue.ap[0]), [H, TPB],
                                 [1, H], [0, D + 1]]))

                    m1_ps = m1_pool.tile([P, RW], F32, space="PSUM", tag="m1")
                    for t in range(TPB):
                        nc.tensor.matmul(
                            m1_ps[:],
                            _ap(eqb.tensor, eqb.offset + t * P,
                                [list(eqb.ap[0]), [1, P]]),
                            _ap(rhs.tensor, rhs.offset + t * RW,
                                [list(rhs.ap[0]), [1, RW]]),
                            start=(t == 0), stop=(t == TPB - 1))

                    # ---- block post ----
                    m1_t = m1_ps.tensor
                    rcp = psb_pool.tile([P, H], F32, tag="rcp")
                    nc.vector.tensor_scalar_add(
                        out=rcp[:],
                        in0=_ap(m1_t, m1_ps.offset + D,
                                [list(m1_ps.ap[0]), [D + 1, H]]),
                        scalar1=1e-16)
                    nc.vector.reciprocal(out=rcp[:], in_=rcp[:])
                    m1n = psb_pool.tile([P, H * D], BF16, tag="m1n")
                    nc.vector.tensor_mul(
                        out=_ap(m1n.tensor, m1n.offset,
                                [list(m1n.ap[0]), [D, H], [1, D]]),
                        in0=_ap(m1_t, m1_ps.offset,
                                [list(m1_ps.ap[0]), [D + 1, H], [1, D]]),
                        in1=_ap(rcp.tensor, rcp.offset,
                                [list(rcp.ap[0]), [1, H], [0, D]]))
                    f_ps = pps_pool.tile([P, D], F32, space="PSUM", tag="fps")
                    for ch in range(N_CH):
                        tp = pps_pool.tile([P, P], BF16, space="PSUM",
                                           tag="tp")
                        nc.tensor.transpose(
                            tp[:], m1n[:, ch * P:(ch + 1) * P], ident_sb)
                        tps = psb_pool.tile([P, P], BF16, tag="tps")
                        nc.any.tensor_copy(out=tps[:], in_=tp[:])
                        nc.tensor.matmul(f_ps[:], tps[:],
                                         wwl_sb[:, ch * D:(ch + 1) * D],
                                         start=(ch == 0), stop=False)
                    nc.tensor.matmul(f_ps[:], ones_sb[:], blp_sb,
                                     start=False, stop=True)
                    f_sb = fout_pool.tile([P, D], F32, tag="fsb")
                    nc.any.tensor_copy(out=f_sb[:], in_=f_ps[:])
                    nc.sync.dma_start(out=out[b * P:(b + 1) * P, :],
                                      in_=f_sb[:])

    nc.compile()
    return nc


def _host_prep(x, edge_index, W, att_src, att_dst, bias, Wl, bl):
    # fused weights (float64 for clean folding)
    Wf = np.asarray(W, np.float64)
    Wlf = np.asarray(Wl, np.float64)
    Was = np.stack([Wf[:, h * D:(h + 1) * D]
                    @ np.asarray(att_src[h], np.float64)
                    for h in range(H)], axis=1)
    Wad = np.stack([Wf[:, h * D:(h + 1) * D]
                    @ np.asarray(att_dst[h], np.float64)
                    for h in range(H)], axis=1)
    Wa = np.concatenate([Was, Wad], axis=1)               # [64, 8]
    WWl_full = np.concatenate(
        [Wf[:, h * D:(h + 1) * D] @ Wlf[h * D:(h + 1) * D, :]
         for h in range(H)], axis=0)                      # [256, 64]
    WWl = np.concatenate([WWl_full[ch * P:(ch + 1) * P, :]
                          for ch in range(N_CH)], axis=1)  # [128, 128]
    blp = (np.asarray(bias, np.float64) @ Wlf
           + np.asarray(bl, np.float64))                  # [64]

    consts = np.zeros((P, 456), NP_BF16)
    consts[:, 0:P] = np.eye(P, dtype=NP_BF16)
    consts[:, P:2 * P] = np.tile(
        np.arange(P, dtype=np.float32).astype(NP_BF16), (P, 1))
    consts[0:D, 256:264] = Wa.astype(NP_BF16)
    consts[:, 264:392] = WWl.astype(NP_BF16)
    consts[0:1, 392:456] = blp.reshape(1, D).astype(NP_BF16)

    # edge tables: sort by dst, then group each block's edges by src window
    src = np.concatenate([np.asarray(edge_index[0]),
                          np.arange(N, dtype=np.int64)]).astype(np.int64)
    dst = np.concatenate([np.asarray(edge_index[1]),
                          np.arange(N, dtype=np.int64)]).astype(np.int64)
    order = np.argsort(dst, kind="stable")
    src = src[order]
    dst = dst[order]
    blk = dst >> 7
    win = src >> 15
    key = blk * N_WIN + win
    order2 = np.argsort(key, kind="stable")
    src, dst, key, win, blk = (src[order2], dst[order2], key[order2],
                               win[order2], blk[order2])
    run_cnt = np.bincount(key, minlength=NT * N_WIN)
    T_W = max(1, int(np.max((run_cnt + P - 1) // P)))
    TPB = N_WIN * T_W
    run_starts = np.zeros(len(run_cnt) + 1, np.int64)
    np.cumsum(run_cnt, out=run_starts[1:])

    jr = np.arange(len(dst), dtype=np.int64) - run_starts[key]
    t_loc = win * T_W + jr // P                    # tile within block
    p = jr % P
    core = (blk // BLOCKS).astype(np.int64)
    b_loc = (blk % BLOCKS).astype(np.int64)

    # flat slot i = t*128 + p within each block's TPB*128 slots
    sv = np.zeros((N_CORES, BLOCKS, TPB * P), np.int64)
    sv[core, b_loc, t_loc * P + p] = src - win * WIN
    dl8 = np.full((N_CORES, BLOCKS * P, TPB), 255.0, np.float32)
    dl8[core, b_loc * P + p, t_loc] = (dst & 127).astype(np.float32)
    dl8 = dl8.astype(NP_BF16)

    def wrap16(v):     # [C, B, TPB*128] -> [C, B*128, TPB*8]
        a = v.reshape(N_CORES, BLOCKS, TPB, 8, 16).astype(np.int16)
        a = a.transpose(0, 1, 4, 2, 3).reshape(N_CORES, BLOCKS, 16, TPB * 8)
        a = np.tile(a, (1, 1, 8, 1))
        return a.reshape(N_CORES, BLOCKS * P, TPB * 8)

    src16 = wrap16(sv)

    x_np = np.asarray(x, np.float32)
    x_ext = np.zeros((N_PAD, XW), NP_BF16)
    x_ext[:N, :D] = x_np.astype(NP_BF16)
    x_ext[:, ONE_COL] = np.float32(1.0).astype(NP_BF16)
    xT = np.zeros((D, N_PAD), NP_BF16)
    xT[:, :N] = x_np.T.astype(NP_BF16)

    shared = {"x_ext": x_ext, "xT": xT, "consts": consts}
    percore = []
    for c in range(N_CORES):
        percore.append({
            "src16": src16[c], "dloc": dl8[c],
            "xTd": np.ascontiguousarray(xT[:, c * NLOC:(c + 1) * NLOC]),
        })
    return shared, percore, T_W


_PROG_CACHE = {}
LAST_EXEC_NS = None


def _run_pjrt(nc, in_maps, n_cores, bench_iters=0):
    """Execute via PJRT (axon) with pre-sharded device buffers; optionally
    re-run for wall-clock timing."""
    import time
    import jax
    from jax.experimental.shard_map import shard_map
    from jax.sharding import Mesh, PartitionSpec, NamedSharding
    from concourse import bass2jax, mybir as mb

    bass2jax.install_neuronx_cc_hook()
    partition_name = (nc.partition_id_tensor.name
                      if nc.partition_id_tensor else None)

    in_names, out_names, out_avals, zero_outs = [], [], [], []
    for alloc in nc.m.functions[0].allocations:
        if not isinstance(alloc, mb.MemoryLocationSet):
            continue
        name = alloc.memorylocations[0].name
        if alloc.kind == "ExternalInput":
            if name != partition_name:
                in_names.append(name)
        elif alloc.kind == "ExternalOutput":
            shape = tuple(alloc.tensor_shape)
            dtype = mb.dt.np(alloc.dtype)
            out_names.append(name)
            out_avals.append(jax.core.ShapedArray(shape, dtype))
            zero_outs.append(np.zeros(shape, dtype))
    n_params = len(in_names)
    all_in_names = in_names + out_names + ([partition_name]
                                           if partition_name else [])

    def _body(*args):
        operands = list(args)
        if partition_name is not None:
            operands.append(bass2jax.partition_id_tensor())
        outs = bass2jax._bass_exec_p.bind(
            *operands,
            out_avals=tuple(out_avals),
            in_names=tuple(all_in_names),
            out_names=tuple(out_names),
            lowering_input_output_aliases=(),
            sim_require_finite=True,
            sim_require_nnan=True,
            nc=nc,
        )
        return tuple(outs)

    devices = jax.devices()[:n_cores]
    mesh = Mesh(np.asarray(devices), ("core",))
    n_outs = len(out_names)
    sharded = jax.jit(
        shard_map(_body, mesh=mesh,
                  in_specs=(PartitionSpec("core"),) * (n_params + n_outs),
                  out_specs=(PartitionSpec("core"),) * n_outs,
                  check_rep=False),
        keep_unused=True,
    )
    concat_in = [
        np.concatenate([np.asarray(in_maps[c][nm]) for c in range(n_cores)],
                       axis=0)
        for nm in in_names
    ]
    concat_zeros = [np.zeros((n_cores * z.shape[0], *z.shape[1:]), z.dtype)
                    for z in zero_outs]
    shard = NamedSharding(mesh, PartitionSpec("core"))
    dev_args = [jax.device_put(a, shard)
                for a in (*concat_in, *concat_zeros)]
    out_arrs = sharded(*dev_args)
    jax.block_until_ready(out_arrs)

    best_ns = None
    if bench_iters:
        times = []
        for _ in range(bench_iters):
            t0 = time.perf_counter_ns()
            r = sharded(*dev_args)
            jax.block_until_ready(r)
            times.append(time.perf_counter_ns() - t0)
        best_ns = min(times)
        print(f"[bench] wall ns per launch: min={min(times)} "
              f"med={sorted(times)[len(times)//2]} max={max(times)}",
              flush=True)

    results = [
        {nm: np.asarray(out_arrs[i]).reshape(n_cores, *out_avals[i].shape)[c]
         for i, nm in enumerate(out_names)}
        for c in range(n_cores)
    ]
    return results, best_ns


def kernel(x, edge_index, W, att_src, att_dst, bias, Wl, bl):
    global LAST_EXEC_NS
    shared, percore, T_W = _host_prep(
        x, edge_index, W, att_src, att_dst, bias, Wl, bl)

    if T_W not in _PROG_CACHE:
        _PROG_CACHE[T_W] = build_program(T_W)
    nc = _PROG_CACHE[T_W]

    in_maps = [dict(shared, **percore[c]) for c in range(N_CORES)]

    if os.environ.get("BASS_GAT_SIM"):
        from concourse.bass_interp import CoreSim
        outs = []
        for c in range(int(os.environ.get("BASS_GAT_SIM_CORES", N_CORES))):
            sim = CoreSim(nc)
            for k, v in in_maps[c].items():
                sim.tensor(k)[:] = v
            sim.simulate()
            outs.append(np.array(sim.tensor("out")))
        while len(outs) < N_CORES:
            outs.append(np.zeros((NLOC, D), np.float32))
    else:
        bench = int(os.environ.get("BASS_GAT_BENCH", "10"))
        results, best_ns = _run_pjrt(nc, in_maps, N_CORES, bench_iters=bench)
        outs = [r["out"] for r in results]
        LAST_EXEC_NS = best_ns
    full = np.concatenate(outs, axis=0)[:N]
    return np.ascontiguousarray(full.astype(np.float32))


# revision 16
# speedup vs baseline: 1.5506x; 1.0250x over previous
"""GAT multi-head block on 8 Trainium2 NeuronCores — v3.

Edge-parallel, dst-sharded. Host sorts the E+N edges (self-loops added) by
dst, shards dst blocks of 128 nodes across cores (98 blocks/core), groups
each block's edges by src int16-window (dma_gather index limit), pads each
(block, window) run to T_W tiles of 128 slots. Fused host weights:
  Wa = [W_h@att_src_h | W_h@att_dst_h] (64x8), WWl_h = W_h@Wl_h,
  blp = bias@Wl + bl.
Device phase A: A = x @ Wa; a_src written into x_ext rows (cols 66:70),
a_dst for the core's own nodes into A_loc [NLOC, 4].
Device phase B per block: one dma_gather per window for x_ext rows (256B,
[x|1|asrc]); one-hot eq[e,v]=(dst_local==v) block-batched; per-edge a_dst
computed on the PE (per tile: transpose eq -> eqT, matmul eqT @ a_dst_blk
into PSUM) — no per-edge a_dst gather, halving SWDGE descriptor-gen which
is the ~7.6ns/row bottleneck; u = exp(leakyrelu(asrc+adst)) block-batched;
rhs = concat_h(u_h*[x|1]) block-batched (4D APs); one PE matmul per tile
accumulates per-head weighted x-sums + softmax denominators in PSUM. Per
block: normalize, transpose, fused WWl_h + bias, write out. Inputs are
device_put with NamedSharding (no per-launch resharding).
"""

import os
import sys
import numpy as np

for _p in ("/opt/trn_rl_repo",):
    if _p not in sys.path:
        sys.path.insert(0, _p)

import concourse.bass as bass
import concourse.bacc as bacc
import concourse.mybir as mybir
import concourse.tile as tile

F32 = mybir.dt.float32
BF16 = mybir.dt.bfloat16
I16 = mybir.dt.int16
NP_BF16 = np.dtype(mybir.dt.np(BF16))

NEG_SLOPE = 0.2
P = 128
N_CORES = 8
D = 64
H = 4
N = 100000
NBLK_TOTAL = (N + P - 1) // P                     # 782
BLOCKS = (NBLK_TOTAL + N_CORES - 1) // N_CORES    # 98
N_PAD = BLOCKS * N_CORES * P                      # 100352
NT = N_PAD // P                                   # 784
NLOC = BLOCKS * P                                 # 12544
WIN = 32768
N_WIN = (N_PAD + WIN - 1) // WIN                  # 4
XW = 128                                          # 256B gather rows
ONE_COL = D                                       # 64
ASRC_COL = 66
RW = H * (D + 1)                                  # 260
N_CH = H * D // P                                 # 2


def _ap(t, offset_elems, dims):
    return bass.AP(t, offset_elems, [list(d) for d in dims])


def build_program(T_W):
    TPB = N_WIN * T_W

    nc = bacc.Bacc("TRN2", target_bir_lowering=False, debug=False,
                   num_devices=N_CORES)

    x_ext = nc.declare_dram_parameter("x_ext", [N_PAD, XW], BF16,
                                      isOutput=False)
    xT = nc.declare_dram_parameter("xT", [D, N_PAD], BF16, isOutput=False)
    xTd = nc.declare_dram_parameter("xTd", [D, NLOC], BF16, isOutput=False)
    # consts [128, 456]: ident | iota | Wa(rows 0:64) | WWl | blp(row 0)
    C_IDENT, C_IOTA, C_WA, C_WWL, C_BLP = 0, 128, 256, 264, 392
    consts = nc.declare_dram_parameter("consts", [P, 456], BF16,
                                       isOutput=False)
    src16 = nc.declare_dram_parameter("src16", [BLOCKS * P, TPB * 8], I16,
                                      isOutput=False)
    dloc = nc.declare_dram_parameter("dloc", [BLOCKS * P, TPB], BF16,
                                     isOutput=False)
    out = nc.declare_dram_parameter("out", [NLOC, D], F32, isOutput=True)
    A_loc = nc.dram_tensor("A_loc", [NLOC, H], BF16)

    A_SLAB = 64
    n_slabs = (NT + A_SLAB - 1) // A_SLAB

    with tile.TileContext(nc) as tc:
        with tc.tile_pool(name="const", bufs=1) as cpool:
            c_sb = cpool.tile([P, 456], BF16, tag="consts")
            nc.sync.dma_start(out=c_sb[:], in_=consts[:])
            ones_sb = cpool.tile([1, P], BF16, tag="ones")
            nc.vector.memset(ones_sb[:], 1.0)
            ident_sb = c_sb[:, C_IDENT:C_IDENT + P]
            wa_sb = c_sb[0:D, C_WA:C_WA + 2 * H]
            wwl_sb = c_sb[:, C_WWL:C_WWL + N_CH * D]
            blp_sb = c_sb[0:1, C_BLP:C_BLP + D]

            # ---------- phase A: [a_src | a_dst] = x @ Wa ----------
            with (
                tc.tile_pool(name="a_xt", bufs=2) as xt_pool,
                tc.tile_pool(name="a_ps", bufs=4, space="PSUM") as aps_pool,
                tc.tile_pool(name="a_st", bufs=2) as ast_pool,
            ):
                # A1: a_src for all nodes -> x_ext cols 66:70
                for s in range(n_slabs):
                    t0 = s * A_SLAB
                    nt = min(A_SLAB, NT - t0)
                    slab = xt_pool.tile([D, A_SLAB * P], BF16, tag="slab")
                    nc.sync.dma_start(out=slab[:, : nt * P],
                                      in_=xT[:, t0 * P:(t0 + nt) * P])
                    stage = ast_pool.tile([P, A_SLAB, 2 * H], BF16, tag="ast")
                    for t in range(nt):
                        aps = aps_pool.tile([P, 2 * H], F32, space="PSUM",
                                            tag="aps")
                        nc.tensor.matmul(aps[:], slab[:, t * P:(t + 1) * P],
                                         wa_sb, start=True, stop=True)
                        nc.any.tensor_copy(out=stage[:, t, :], in_=aps[:])
                    nc.sync.dma_start(
                        out=_ap(x_ext, t0 * P * XW + ASRC_COL,
                                [[XW, P], [P * XW, nt], [1, H]]),
                        in_=stage[:, :nt, 0:H])
                # A2: a_dst for this core's own nodes -> A_loc cols 0:4
                NTd = NLOC // P
                n_slabs_d = (NTd + A_SLAB - 1) // A_SLAB
                for s in range(n_slabs_d):
                    t0 = s * A_SLAB
                    nt = min(A_SLAB, NTd - t0)
                    slab = xt_pool.tile([D, A_SLAB * P], BF16, tag="slab")
                    nc.sync.dma_start(out=slab[:, : nt * P],
                                      in_=xTd[:, t0 * P:(t0 + nt) * P])
                    stage = ast_pool.tile([P, A_SLAB, 2 * H], BF16, tag="ast")
                    for t in range(nt):
                        aps = aps_pool.tile([P, 2 * H], F32, space="PSUM",
                                            tag="aps")
                        nc.tensor.matmul(aps[:], slab[:, t * P:(t + 1) * P],
                                         wa_sb, start=True, stop=True)
                        nc.any.tensor_copy(out=stage[:, t, :], in_=aps[:])
                    nc.sync.dma_start(
                        out=_ap(A_loc, t0 * P * H,
                                [[H, P], [P * H, nt], [1, H]]),
                        in_=stage[:, :nt, H:2 * H])

            # ---------- phase B ----------
            with (
                tc.tile_pool(name="idx", bufs=4) as idx_pool,
                tc.tile_pool(name="gx", bufs=4) as gx_pool,
                tc.tile_pool(name="adb", bufs=4) as adb_pool,
                tc.tile_pool(name="uexp", bufs=3) as u_pool,
                tc.tile_pool(name="eq", bufs=3) as eq_pool,
                tc.tile_pool(name="eqt", bufs=4) as eqt_pool,
                tc.tile_pool(name="eqt_ps", bufs=2, space="PSUM") as etp_pool,
                tc.tile_pool(name="ad_ps", bufs=2, space="PSUM") as adp_pool,
                tc.tile_pool(name="rhs", bufs=3) as rhs_pool,
                tc.tile_pool(name="m1", bufs=2, space="PSUM") as m1_pool,
                tc.tile_pool(name="post_ps", bufs=1, space="PSUM") as pps_pool,
                tc.tile_pool(name="post_sb", bufs=3) as psb_pool,
                tc.tile_pool(name="fout", bufs=2) as fout_pool,
            ):
                for b in range(BLOCKS):
                    s_sb = idx_pool.tile([P, TPB * 8], I16, tag="s16")
                    nc.sync.dma_start(out=s_sb[:],
                                      in_=src16[b * P:(b + 1) * P, :])
                    dl_sb = idx_pool.tile([P, TPB], BF16, tag="dl")
                    nc.sync.dma_start(out=dl_sb[:],
                                      in_=dloc[b * P:(b + 1) * P, :])
                    adb = adb_pool.tile([P, H], BF16, tag="adb")
                    nc.sync.dma_start(out=adb[:],
                                      in_=A_loc[b * P:(b + 1) * P, :])

                    gxb = gx_pool.tile([P, TPB, XW], BF16, tag="gx")
                    for w in range(N_WIN):
                        rows_w = min(WIN, N_PAD - w * WIN)
                        nc.gpsimd.dma_gather(
                            gxb[:, w * T_W:(w + 1) * T_W, :],
                            _ap(x_ext, w * WIN * XW,
                                [[XW, rows_w], [1, XW]]),
                            s_sb[:, w * T_W * 8:(w + 1) * T_W * 8],
                            T_W * P, T_W * P, XW, single_packet=False)

                    # eq[p, t, v] = (dl[p, t] == v), all tiles at once
                    eqb = eq_pool.tile([P, TPB, P], BF16, tag="eqb")
                    nc.vector.tensor_tensor(
                        out=eqb[:],
                        in0=_ap(dl_sb.tensor, dl_sb.offset,
                                [list(dl_sb.ap[0]), [1, TPB], [0, P]]),
                        in1=_ap(c_sb.tensor, c_sb.offset + C_IOTA,
                                [list(c_sb.ap[0]), [0, TPB], [1, P]]),
                        op=mybir.AluOpType.is_equal)

                    # per-edge a_dst on PE: adst[e, h] = sum_v eqT[v,e] adb[v,h]
                    ad_ps = adp_pool.tile([P, TPB, H], F32, space="PSUM",
                                          tag="adps")
                    for t in range(TPB):
                        etp = etp_pool.tile([P, P], BF16, space="PSUM",
                                            tag="etp")
                        nc.tensor.transpose(
                            etp[:],
                            _ap(eqb.tensor, eqb.offset + t * P,
                                [list(eqb.ap[0]), [1, P]]),
                            ident_sb)
                        eqt = eqt_pool.tile([P, P], BF16, tag="eqt")
                        nc.any.tensor_copy(out=eqt[:], in_=etp[:])
                        nc.tensor.matmul(ad_ps[:, t, :], eqt[:], adb[:],
                                         start=True, stop=True)

                    # u = exp(leaky_relu(asrc + adst)) for the whole block
                    lg = u_pool.tile([P, TPB, H], F32, tag="lg")
                    nc.vector.tensor_add(
                        out=lg[:],
                        in0=_ap(gxb.tensor, gxb.offset + ASRC_COL,
                                [list(gxb.ap[0]), [XW, TPB], [1, H]]),
                        in1=ad_ps[:])
                    lr = u_pool.tile([P, TPB, H], F32, tag="lr")
                    nc.vector.scalar_tensor_tensor(
                        out=lr[:], in0=lg[:], scalar=NEG_SLOPE, in1=lg[:],
                        op0=mybir.AluOpType.mult, op1=mybir.AluOpType.max)
                    ue = u_pool.tile([P, TPB, H], BF16, tag="ue")
                    nc.scalar.activation(out=ue[:], in_=lr[:],
                                         func=mybir.ActivationFunctionType.Exp)
                    # rhs[p, t, h, c] = gx[p, t, c] * ue[p, t, h], c in 0..64
                    rhs = rhs_pool.tile([P, TPB, RW], BF16, tag="rhs")
                    nc.vector.tensor_mul(
                        out=_ap(rhs.tensor, rhs.offset,
                                [list(rhs.ap[0]), [RW, TPB],
                                 [D + 1, H], [1, D + 1]]),
                        in0=_ap(gxb.tensor, gxb.offset,
                                [list(gxb.ap[0]), [XW, TPB],
                                 [0, H], [1, D + 1]]),
                        in1=_ap(ue.tensor, ue.offset,
                                [list(ue.ap[0]), [H, TPB],
                                 [1, H], [0, D + 1]]))

                    m1_ps = m1_pool.tile([P, RW], F32, space="PSUM", tag="m1")
                    for t in range(TPB):
                        nc.tensor.matmul(
                            m1_ps[:],
                            _ap(eqb.tensor, eqb.offset + t * P,
                                [list(eqb.ap[0]), [1, P]]),
                            _ap(rhs.tensor, rhs.offset + t * RW,
                                [list(rhs.ap[0]), [1, RW]]),
                            start=(t == 0), stop=(t == TPB - 1))

                    # ---- block post ----
                    m1_t = m1_ps.tensor
                    rcp = psb_pool.tile([P, H], F32, tag="rcp")
                    nc.vector.tensor_scalar_add(
                        out=rcp[:],
                        in0=_ap(m1_t, m1_ps.offset + D,
                                [list(m1_ps.ap[0]), [D + 1, H]]),
                        scalar1=1e-16)
                    nc.vector.reciprocal(out=rcp[:], in_=rcp[:])
                    m1n = psb_pool.tile([P, H * D], BF16, tag="m1n")
                    nc.vector.tensor_mul(
                        out=_ap(m1n.tensor, m1n.offset,
                                [list(m1n.ap[0]), [D, H], [1, D]]),
                        in0=_ap(m1_t, m1_ps.offset,
                                [list(m1_ps.ap[0]), [D + 1, H], [1, D]]),
                        in1=_ap(rcp.tensor, rcp.offset,
                                [list(rcp.ap[0]), [1, H], [0, D]]))
                    f_ps = pps_pool.tile([P, D], F32, space="PSUM", tag="fps")
                    for ch in range(N_CH):
                        tp = pps_pool.tile([P, P], BF16, space="PSUM",
                                           tag="tp")
                        nc.tensor.transpose(
                            tp[:], m1n[:, ch * P:(ch + 1) * P], ident_sb)
                        tps = psb_pool.tile([P, P], BF16, tag="tps")
                        nc.any.tensor_copy(out=tps[:], in_=tp[:])
                        nc.tensor.matmul(f_ps[:], tps[:],
                                         wwl_sb[:, ch * D:(ch + 1) * D],
                                         start=(ch == 0), stop=False)
                    nc.tensor.matmul(f_ps[:], ones_sb[:], blp_sb,
                                     start=False, stop=True)
                    f_sb = fout_pool.tile([P, D], F32, tag="fsb")
                    nc.any.tensor_copy(out=f_sb[:], in_=f_ps[:])
                    nc.sync.dma_start(out=out[b * P:(b + 1) * P, :],
                                      in_=f_sb[:])

    nc.compile()
    return nc


def _host_prep(x, edge_index, W, att_src, att_dst, bias, Wl, bl):
    # fused weights (float64 for clean folding)
    Wf = np.asarray(W, np.float64)
    Wlf = np.asarray(Wl, np.float64)
    Was = np.stack([Wf[:, h * D:(h + 1) * D]
                    @ np.asarray(att_src[h], np.float64)
                    for h in range(H)], axis=1)
    Wad = np.stack([Wf[:, h * D:(h + 1) * D]
                    @ np.asarray(att_dst[h], np.float64)
                    for h in range(H)], axis=1)
    Wa = np.concatenate([Was, Wad], axis=1)               # [64, 8]
    WWl_full = np.concatenate(
        [Wf[:, h * D:(h + 1) * D] @ Wlf[h * D:(h + 1) * D, :]
         for h in range(H)], axis=0)                      # [256, 64]
    WWl = np.concatenate([WWl_full[ch * P:(ch + 1) * P, :]
                          for ch in range(N_CH)], axis=1)  # [128, 128]
    blp = (np.asarray(bias, np.float64) @ Wlf
           + np.asarray(bl, np.float64))                  # [64]

    consts = np.zeros((P, 456), NP_BF16)
    consts[:, 0:P] = np.eye(P, dtype=NP_BF16)
    consts[:, P:2 * P] = np.tile(
        np.arange(P, dtype=np.float32).astype(NP_BF16), (P, 1))
    consts[0:D, 256:264] = Wa.astype(NP_BF16)
    consts[:, 264:392] = WWl.astype(NP_BF16)
    consts[0:1, 392:456] = blp.reshape(1, D).astype(NP_BF16)

    # edge tables: sort by dst, then group each block's edges by src window
    src = np.concatenate([np.asarray(edge_index[0]),
                          np.arange(N, dtype=np.int64)]).astype(np.int64)
    dst = np.concatenate([np.asarray(edge_index[1]),
                          np.arange(N, dtype=np.int64)]).astype(np.int64)
    order = np.argsort(dst, kind="stable")
    src = src[order]
    dst = dst[order]
    blk = dst >> 7
    win = src >> 15
    key = blk * N_WIN + win
    order2 = np.argsort(key, kind="stable")
    src, dst, key, win, blk = (src[order2], dst[order2], key[order2],
                               win[order2], blk[order2])
    run_cnt = np.bincount(key, minlength=NT * N_WIN)
    T_W = max(1, int(np.max((run_cnt + P - 1) // P)))
    TPB = N_WIN * T_W
    run_starts = np.zeros(len(run_cnt) + 1, np.int64)
    np.cumsum(run_cnt, out=run_starts[1:])

    jr = np.arange(len(dst), dtype=np.int64) - run_starts[key]
    t_loc = win * T_W + jr // P                    # tile within block
    p = jr % P
    core = (blk // BLOCKS).astype(np.int64)
    b_loc = (blk % BLOCKS).astype(np.int64)

    # flat slot i = t*128 + p within each block's TPB*128 slots
    sv = np.zeros((N_CORES, BLOCKS, TPB * P), np.int64)
    sv[core, b_loc, t_loc * P + p] = src - win * WIN
    dl8 = np.full((N_CORES, BLOCKS * P, TPB), 255.0, np.float32)
    dl8[core, b_loc * P + p, t_loc] = (dst & 127).astype(np.float32)
    dl8 = dl8.astype(NP_BF16)

    def wrap16(v):     # [C, B, TPB*128] -> [C, B*128, TPB*8]
        a = v.reshape(N_CORES, BLOCKS, TPB, 8, 16).astype(np.int16)
        a = a.transpose(0, 1, 4, 2, 3).reshape(N_CORES, BLOCKS, 16, TPB * 8)
        a = np.tile(a, (1, 1, 8, 1))
        return a.reshape(N_CORES, BLOCKS * P, TPB * 8)

    src16 = wrap16(sv)

    x_np = np.asarray(x, np.float32)
    x_ext = np.zeros((N_PAD, XW), NP_BF16)
    x_ext[:N, :D] = x_np.astype(NP_BF16)
    x_ext[:, ONE_COL] = np.float32(1.0).astype(NP_BF16)
    xT = np.zeros((D, N_PAD), NP_BF16)
    xT[:, :N] = x_np.T.astype(NP_BF16)

    shared = {"x_ext": x_ext, "xT": xT, "consts": consts}
    percore = []
    for c in range(N_CORES):
        percore.append({
            "src16": src16[c], "dloc": dl8[c],
            "xTd": np.ascontiguousarray(xT[:, c * NLOC:(c + 1) * NLOC]),
        })
    return shared, percore, T_W


_PROG_CACHE = {}
LAST_EXEC_NS = None


def _run_pjrt(nc, in_maps, n_cores, bench_iters=0):
    """Execute via PJRT (axon) with pre-sharded device buffers; optionally
    re-run for wall-clock timing."""
    import time
    import jax
    from jax.experimental.shard_map import shard_map
    from jax.sharding import Mesh, PartitionSpec, NamedSharding
    from concourse import bass2jax, mybir as mb

    bass2jax.install_neuronx_cc_hook()
    partition_name = (nc.partition_id_tensor.name
                      if nc.partition_id_tensor else None)

    in_names, out_names, out_avals, zero_outs = [], [], [], []
    for alloc in nc.m.functions[0].allocations:
        if not isinstance(alloc, mb.MemoryLocationSet):
            continue
        name = alloc.memorylocations[0].name
        if alloc.kind == "ExternalInput":
            if name != partition_name:
                in_names.append(name)
        elif alloc.kind == "ExternalOutput":
            shape = tuple(alloc.tensor_shape)
            dtype = mb.dt.np(alloc.dtype)
            out_names.append(name)
            out_avals.append(jax.core.ShapedArray(shape, dtype))
            zero_outs.append(np.zeros(shape, dtype))
    n_params = len(in_names)
    all_in_names = in_names + out_names + ([partition_name]
                                           if partition_name else [])

    def _body(*args):
        operands = list(args)
        if partition_name is not None:
            operands.append(bass2jax.partition_id_tensor())
        outs = bass2jax._bass_exec_p.bind(
            *operands,
            out_avals=tuple(out_avals),
            in_names=tuple(all_in_names),
            out_names=tuple(out_names),
            lowering_input_output_aliases=(),
            sim_require_finite=True,
            sim_require_nnan=True,
            nc=nc,
        )
        return tuple(outs)

    devices = jax.devices()[:n_cores]
    mesh = Mesh(np.asarray(devices), ("core",))
    n_outs = len(out_names)
    sharded = jax.jit(
        shard_map(_body, mesh=mesh,
                  in_specs=(PartitionSpec("core"),) * (n_params + n_outs),
                  out_specs=(PartitionSpec("core"),) * n_outs,
                  check_rep=False),
        keep_unused=True,
    )
    concat_in = [
        np.concatenate([np.asarray(in_maps[c][nm]) for c in range(n_cores)],
                       axis=0)
        for nm in in_names
    ]
    concat_zeros = [np.zeros((n_cores * z.shape[0], *z.shape[1:]), z.dtype)
                    for z in zero_outs]
    shard = NamedSharding(mesh, PartitionSpec("core"))
    dev_args = [jax.device_put(a, shard)
                for a in (*concat_in, *concat_zeros)]
    out_arrs = sharded(*dev_args)
    jax.block_until_ready(out_arrs)

    best_ns = None
    if bench_iters:
        times = []
        for _ in range(bench_iters):
            t0 = time.perf_counter_ns()
            r = sharded(*dev_args)
            jax.block_until_ready(r)
            times.append(time.perf_counter_ns() - t0)
        best_ns = min(times)
        print(f"[bench] wall ns per launch: min={min(times)} "
              f"med={sorted(times)[len(times)//2]} max={max(times)}",
              flush=True)

    results = [
        {nm: np.asarray(out_arrs[i]).reshape(n_cores, *out_avals[i].shape)[c]
         for i, nm in enumerate(out_names)}
        for c in range(n_cores)
    ]
    return results, best_ns


def kernel(x, edge_index, W, att_src, att_dst, bias, Wl, bl):
    global LAST_EXEC_NS
    shared, percore, T_W = _host_prep(
        x, edge_index, W, att_src, att_dst, bias, Wl, bl)

    if T_W not in _PROG_CACHE:
        _PROG_CACHE[T_W] = build_program(T_W)
    nc = _PROG_CACHE[T_W]

    in_maps = [dict(shared, **percore[c]) for c in range(N_CORES)]

    if os.environ.get("BASS_GAT_SIM"):
        from concourse.bass_interp import CoreSim
        outs = []
        for c in range(int(os.environ.get("BASS_GAT_SIM_CORES", N_CORES))):
            sim = CoreSim(nc)
            for k, v in in_maps[c].items():
                sim.tensor(k)[:] = v
            sim.simulate()
            outs.append(np.array(sim.tensor("out")))
        while len(outs) < N_CORES:
            outs.append(np.zeros((NLOC, D), np.float32))
    else:
        bench = int(os.environ.get("BASS_GAT_BENCH", "10"))
        results, best_ns = _run_pjrt(nc, in_maps, N_CORES, bench_iters=bench)
        outs = [r["out"] for r in results]
        LAST_EXEC_NS = best_ns
    full = np.concatenate(outs, axis=0)[:N]
    return np.ascontiguousarray(full.astype(np.float32))
